# revision 13
# baseline (speedup 1.0000x reference)
"""Trainium2 Bass kernel for the controlled-U (CU) gate application.

Math: the reference builds U = P0 (x) I (x) ... + P1 (x) Mexp (x) I ...
with dim=2, wires=12, index=(0,1), control_state=(1,). This factors as

    U = diag(I_2, Mexp) (x) I_1024        (4096 x 4096)

so U @ x is:
    out[0:2048]     = x[0:2048]                        (identity)
    out[2048:3072]  = c00 * x[2048:3072] + c01 * x[3072:4096]
    out[3072:4096]  = c10 * x[2048:3072] + c11 * x[3072:4096]

with [[c00, c01], [c10, c11]] = Mexp = expm(M - M^H), a 2x2 unitary
computed on host (it is a 2x2 matrix; eigendecomposition of the
Hermitian generator gives the exact exponential).

Device strategy (8 NeuronCores, SPMD, batch-column sharding):
  - each core gets a (4096, 128) column shard of x_re / x_im
  - top 2048 rows: DVE strided copies interleave re/im -> complex64 layout
  - bottom 2048 rows: TensorE matmuls with diagonal stationary matrices
    (coefficients are *data*, so one compiled NEFF serves any M), PSUM
    accumulation, ACT engine interleave-copies PSUM -> SBUF
  - output per core: (4096, 256) f32 = interleaved complex; host gathers
    column shards and reinterprets as complex64 (zero-copy view).
"""

import numpy as np

import concourse.bass as bass
import concourse.bacc as bacc
import concourse.mybir as mybir
from concourse.tile import TileContext
from concourse.bass_utils import run_bass_kernel_spmd

# Problem geometry (hardcoded per the task contract).
D = 4096           # state dimension 2**12
B = 1024           # batch
NCORES = 8
BC = B // NCORES   # 128 batch columns per core
P = 128            # SBUF partitions
F32 = mybir.dt.float32
F32R = mybir.dt.float32r

NDIAG = 12         # 12 diagonal coefficient matrices (see _coef_values)


def _build_nc() -> bass.Bass:
    """Build the per-core Bass/Tile program (identical on all 8 cores)."""
    # Bacc (not raw Bass): its compile() lowers multi-dependency sync waits
    # through event semaphores — raw Bass trips walrus's per-instruction
    # wait-slot limit ("Too many sync wait commands").
    nc = bacc.Bacc("TRN2")

    xr = nc.dram_tensor("xr", [D, BC], F32, kind="ExternalInput")
    xi = nc.dram_tensor("xi", [D, BC], F32, kind="ExternalInput")
    coef = nc.dram_tensor("coef", [P, NDIAG * P], F32, kind="ExternalInput")
    out = nc.dram_tensor("out", [D, 2 * BC], F32, kind="ExternalOutput")

    # Row-block views: row = b*128 + p  ->  (p, b, j)
    xr_top = xr[0 : D // 2, :].rearrange("(b p) j -> p b j", p=P)      # (128, 16, 128)
    xi_top = xi[0 : D // 2, :].rearrange("(b p) j -> p b j", p=P)
    out_top = out[0 : D // 2, :].rearrange("(b p) j -> p b j", p=P)    # (128, 16, 256)

    # Bottom half split into h=0 (rows 2048:3072) / h=1 (rows 3072:4096),
    # k = block within half.  Pairing (h=0, h=1) at equal k keeps both
    # operands of the 2x2 mix in one tile.
    xr_bot = xr[D // 2 :, :].rearrange("(h k p) j -> p h k j", h=2, k=8)   # (128,2,8,128)
    xi_bot = xi[D // 2 :, :].rearrange("(h k p) j -> p h k j", h=2, k=8)
    out_bot = out[D // 2 :, :].rearrange("(h k p) j -> p h k j", h=2, k=8)  # (128,2,8,256)

    with TileContext(nc) as tc:
        with (
            tc.tile_pool(name="const", bufs=1) as const_pool,
            tc.tile_pool(name="io", bufs=3) as io_pool,
            tc.tile_pool(name="psum", bufs=7, space="PSUM") as psum_pool,
            tc.tile_pool(name="psum_warm", bufs=1, space="PSUM") as warm_pool,
        ):
            # float32r end-to-end for matmul operands: the BIR verifier
            # requires the producer of an FP32r matmult input to emit f32r.
            coef_sb = const_pool.tile([P, NDIAG * P], F32R)
            nc.sync.dma_start(coef_sb[:], coef[:].bitcast(F32R))

            def cdiag(k: int):
                return coef_sb[:, k * P : (k + 1) * P]

            # PE warmup: a 1-column matmul whose only dependency is the coef
            # DMA.  The S3_LW (ldweights) slot allows very few sync waits, so
            # every subsequent matmul must introduce at most one new
            # dependency; this one "observes" coef_sb for the PE engine.
            warm_ps = warm_pool.tile([P, 2], F32, tag="warm")
            nc.tensor.matmul(warm_ps[:], cdiag(0), coef_sb[:, 0:2],
                             start=True, stop=True)

            # ---- top half: identity, just interleave re/im ----
            TOPG = 8  # blocks per group
            for g in range(16 // TOPG):
                bs = slice(g * TOPG, (g + 1) * TOPG)
                xr_g = io_pool.tile([P, TOPG, BC], F32, tag="xr_top")
                xi_g = io_pool.tile([P, TOPG, BC], F32, tag="xi_top")
                nc.sync.dma_start(xr_g[:], xr_top[:, bs, :])
                nc.sync.dma_start(xi_g[:], xi_top[:, bs, :])
                o_g = io_pool.tile([P, TOPG, 2 * BC], F32, tag="out_top")
                nc.vector.tensor_copy(o_g[:, :, 0 : 2 * BC : 2], xr_g[:])
                nc.vector.tensor_copy(o_g[:, :, 1 : 2 * BC : 2], xi_g[:])
                nc.sync.dma_start(out_top[:, bs, :], o_g[:])

            # ---- bottom half: 2x2 complex mix on TensorE ----
            BOTG = 4  # k-blocks per group
            for g in range(8 // BOTG):
                ks = slice(g * BOTG, (g + 1) * BOTG)
                # one tile per (tensor, half) so each tile has exactly one
                # DMA writer -> each matmul adds at most one new sync wait.
                xr1_t = io_pool.tile([P, BOTG, BC], F32R, tag="xr1")
                xi1_t = io_pool.tile([P, BOTG, BC], F32R, tag="xi1")
                xr2_t = io_pool.tile([P, BOTG, BC], F32R, tag="xr2")
                xi2_t = io_pool.tile([P, BOTG, BC], F32R, tag="xi2")
                nc.sync.dma_start(xr1_t[:], xr_bot[:, 0, ks, :].bitcast(F32R))
                nc.sync.dma_start(xi1_t[:], xi_bot[:, 0, ks, :].bitcast(F32R))
                nc.sync.dma_start(xr2_t[:], xr_bot[:, 1, ks, :].bitcast(F32R))
                nc.sync.dma_start(xi2_t[:], xi_bot[:, 1, ks, :].bitcast(F32R))

                r1 = xr1_t[:]   # (128, 4, 128) free=512
                i1 = xi1_t[:]
                r2 = xr2_t[:]
                i2 = xi2_t[:]

                # accumulation recipes: psum_quantity -> [(diag_idx, moving), ...]
                recipes = {
                    "o1re": [(0, r1), (1, i1), (3, r2), (4, i2)],
                    "o1im": [(2, r1), (0, i1), (5, r2), (3, i2)],
                    "o2re": [(6, r1), (7, i1), (9, r2), (10, i2)],
                    "o2im": [(8, r1), (6, i1), (11, r2), (9, i2)],
                }
                ps = {}
                for name, terms in recipes.items():
                    pt = psum_pool.tile([P, BOTG, BC], F32, tag="ps")
                    for t, (k, mv) in enumerate(terms):
                        nc.tensor.matmul(
                            pt[:], cdiag(k), mv,
                            start=(t == 0), stop=(t == len(terms) - 1),
                        )
                    ps[name] = pt

                o_g = io_pool.tile([P, 2, BOTG, 2 * BC], F32, tag="out_bot")
                nc.scalar.copy(o_g[:, 0, :, 0 : 2 * BC : 2], ps["o1re"][:])
                nc.scalar.copy(o_g[:, 0, :, 1 : 2 * BC : 2], ps["o1im"][:])
                nc.scalar.copy(o_g[:, 1, :, 0 : 2 * BC : 2], ps["o2re"][:])
                nc.scalar.copy(o_g[:, 1, :, 1 : 2 * BC : 2], ps["o2im"][:])
                for h in range(2):
                    nc.sync.dma_start(out_bot[:, h, ks, :], o_g[:, h])

    nc.finalize()
    return nc


_NC_CACHE = None


def _get_nc() -> bass.Bass:
    global _NC_CACHE
    if _NC_CACHE is None:
        _NC_CACHE = _build_nc()
    return _NC_CACHE


def _coef_values(M_re: np.ndarray, M_im: np.ndarray) -> np.ndarray:
    """Host-side 2x2 expm of the anti-Hermitian generator -> 12 diag values."""
    M = M_re.astype(np.float64) + 1j * M_im.astype(np.float64)
    A = M - M.conj().T          # anti-Hermitian
    H = -1j * A                 # Hermitian
    w, V = np.linalg.eigh(H)
    Mexp = V @ np.diag(np.exp(1j * w)) @ V.conj().T   # expm(A), exact
    c00, c01 = Mexp[0, 0], Mexp[0, 1]
    c10, c11 = Mexp[1, 0], Mexp[1, 1]
    vals = [
        c00.real, -c00.imag, c00.imag,
        c01.real, -c01.imag, c01.imag,
        c10.real, -c10.imag, c10.imag,
        c11.real, -c11.imag, c11.imag,
    ]
    coef = np.zeros((P, NDIAG * P), dtype=np.float32)
    idx = np.arange(P)
    for k, v in enumerate(vals):
        coef[idx, k * P + idx] = np.float32(v)
    return coef


def kernel(M_re, M_im, x_re, x_im) -> np.ndarray:
    M_re = np.asarray(M_re, dtype=np.float32)
    M_im = np.asarray(M_im, dtype=np.float32)
    x_re = np.asarray(x_re, dtype=np.float32)
    x_im = np.asarray(x_im, dtype=np.float32)

    coef = _coef_values(M_re, M_im)

    in_maps = []
    for d in range(NCORES):
        cols = slice(d * BC, (d + 1) * BC)
        in_maps.append({
            "xr": np.ascontiguousarray(x_re[:, cols]),
            "xi": np.ascontiguousarray(x_im[:, cols]),
            "coef": coef,
        })

    nc = _get_nc()
    res = run_bass_kernel_spmd(nc, in_maps, core_ids=list(range(NCORES)))
    full = np.concatenate([r["out"] for r in res.results], axis=1)  # (4096, 2048) f32
    return full.view(np.complex64)  # (4096, 1024)
